# revision 17
# baseline (speedup 1.0000x reference)
"""Trainium2 Bass kernel: complex-valued transformer block (nn_EqModelComplex).

Sharding: 8 cores = (batch b in 0..3) x (query-half h in 0..1); each core does
512 query tokens of one batch element, K/V over its full 1024-token sequence.
No collectives.

Feature-major (FM) layout: SBUF tiles are (features_on_partitions,
tokens_on_free); host pre-transposes x and weights, transposes output back.
Matmuls in float32r (attention value-path in bf16). Complex linear via one
PSUM per real/imag output using a negated-wi copy:
  yr: sum_k [ wr_k.T @ xr_k + (-wi_k).T @ xi_k ]
  yi: sum_k [ wi_k.T @ xr_k +   wr_k.T @ xi_k ]
Attention: transposed scores (tk on partitions, tq free), exp without
max-subtraction, multiplicative causal mask after exp, softmax denominators
via a ones-column appended to V.
"""
import sys, os
sys.path.insert(0, '/opt/trn_rl_repo')
import math
import numpy as np
from contextlib import ExitStack

P = 128
D = 512
S = 1024
B = 4
H = 8
HD = 64
HID = 2048
TQ = 512
FT = D // P          # 4
KT_HID = HID // P    # 16
NCORES = 8
EPS = 1e-6
SCALE = 1.0 / math.sqrt(HD)

_CACHE = {}


def _emit_body(nc, tc, io):
    from concourse import mybir

    dt = mybir.dt
    AF = mybir.ActivationFunctionType
    ALU = mybir.AluOpType
    f32 = dt.float32
    f32r = dt.float32r
    bf16 = dt.bfloat16
    TT = nc.vector.tensor_tensor
    TS = nc.vector.tensor_scalar
    lnrow = [0]

    ctx = ExitStack()
    with ctx:
        # ---------------- long-lived pools ----------------
        const = ctx.enter_context(tc.tile_pool(name="const", bufs=1))
        p_w = ctx.enter_context(tc.tile_pool(name="p_w", bufs=4))       # wr/wi/wineg keep
        p_wtmp = ctx.enter_context(tc.tile_pool(name="p_wtmp", bufs=1))
        p_bc = ctx.enter_context(tc.tile_pool(name="p_bc", bufs=1))
        ps = ctx.enter_context(tc.tile_pool(name="ps", bufs=6, space="PSUM"))
        ps_at = ctx.enter_context(tc.tile_pool(name="ps_at", bufs=1, space="PSUM"))

        ones_f = const.tile([P, 1], f32)
        nc.vector.memset(ones_f, 1.0)
        ones32r = const.tile([P, 1], f32r)
        nc.vector.tensor_copy(out=ones32r, in_=ones_f)
        c4 = const.tile([P, 1], f32)
        nc.vector.memset(c4, 4.0)
        ceps = const.tile([P, 1], f32)
        nc.vector.memset(ceps, EPS)

        lncols = {}
        for key in ['ln1_gr', 'ln1_gi', 'ln1_br', 'ln1_bi',
                    'ln2_gr', 'ln2_gi', 'ln2_br', 'ln2_bi']:
            c = const.tile([P, FT], f32, name='c_' + key)
            nc.sync.dma_start(out=c, in_=io[key].rearrange("(t p) -> p t", p=P))
            lncols[key] = c

        bias_cols = {}
        for nm in ['q', 'k', 'v', 'o']:
            bm = const.tile([P, FT], f32, name='cb_bm_' + nm)
            bp = const.tile([P, FT], f32, name='cb_bp_' + nm)
            nc.sync.dma_start(out=bm, in_=io[nm + '_bm'].rearrange("(t p) -> p t", p=P))
            nc.sync.dma_start(out=bp, in_=io[nm + '_bp'].rearrange("(t p) -> p t", p=P))
            ssq = const.tile([P, FT], f32, name='cb_ssq_' + nm)
            cosv = const.tile([P, FT], f32, name='cb_cosv_' + nm)
            br_ = const.tile([P, FT], f32, name='bias_r_' + nm)
            bi_ = const.tile([P, FT], f32, name='bias_i_' + nm)
            nc.scalar.activation(out=bp, in_=bp, func=AF.Sin, scale=0.5)   # s
            TT(out=ssq, in0=bp, in1=bp, op=ALU.mult)
            TS(out=cosv, in0=ssq, scalar1=-2.0, scalar2=1.0, op0=ALU.mult, op1=ALU.add)
            nc.scalar.activation(out=ssq, in_=ssq, func=AF.Sqrt, scale=-4.0, bias=c4)  # 2c
            TT(out=bp, in0=bp, in1=ssq, op=ALU.mult)    # sin
            TT(out=br_, in0=bm, in1=cosv, op=ALU.mult)
            TT(out=bi_, in0=bm, in1=bp, op=ALU.mult)
            bias_cols[nm] = (br_, bi_)

        # ---------- weight preprocessing ----------
        def prep_w(lm_ap, ph_ap, width, out_dt=f32r):
            lm = p_wtmp.tile([P, width], f32, tag="wp_lm")
            ph = p_wtmp.tile([P, width], f32, tag="wp_ph")
            nc.sync.dma_start(out=lm, in_=lm_ap)
            nc.sync.dma_start(out=ph, in_=ph_ap)
            ssq = p_wtmp.tile([P, width], f32, tag="wp_ssq")
            cosv = p_wtmp.tile([P, width], f32, tag="wp_cosv")
            wr = p_w.tile([P, width], out_dt, tag="pw_wr")
            wi = p_w.tile([P, width], out_dt, tag="pw_wi")
            wineg = p_w.tile([P, width], out_dt, tag="pw_wineg")
            nc.scalar.activation(out=lm, in_=lm, func=AF.Exp)               # mag
            nc.scalar.activation(out=ph, in_=ph, func=AF.Sin, scale=0.5)    # s
            TT(out=ssq, in0=ph, in1=ph, op=ALU.mult)
            TS(out=cosv, in0=ssq, scalar1=-2.0, scalar2=1.0, op0=ALU.mult, op1=ALU.add)
            nc.scalar.activation(out=ssq, in_=ssq, func=AF.Sqrt, scale=-4.0, bias=c4)  # 2c
            TT(out=ph, in0=ph, in1=ssq, op=ALU.mult)    # sin
            TT(out=wr, in0=lm, in1=cosv, op=ALU.mult)
            TT(out=wi, in0=lm, in1=ph, op=ALU.mult)
            TS(out=wineg, in0=wi, scalar1=-1.0, scalar2=None, op0=ALU.mult)
            return wr, wi, wineg

        def proj_weights(nm):
            lmT, phT = io[nm + '_lmT'], io[nm + '_phT']
            return [prep_w(lmT[kt * P:(kt + 1) * P, :], phT[kt * P:(kt + 1) * P, :], D)
                    for kt in range(FT)]

        # ---------- chunked feature-major layernorm (512 tokens) ----------
        def ln_chunk(xr_t, xi_t, ln, dst_r, dst_i, hpool, htag, tpool, hbufs=5):
            NT = 512
            pssum = [ps.tile([1, NT], f32, tag="ps_rot", name="lnps%d" % q) for q in range(3)]
            for kt in range(FT):
                xr32 = tpool.tile([P, NT], f32r, tag="ln_x32r", bufs=2)
                xi32 = tpool.tile([P, NT], f32r, tag="ln_x32i", bufs=2)
                sq32 = tpool.tile([P, NT], f32r, tag="ln_sq32", bufs=2)
                ta = tpool.tile([P, NT], f32, tag="ln_ta", bufs=2)
                nc.vector.tensor_copy(out=xr32, in_=xr_t[kt])
                nc.vector.tensor_copy(out=xi32, in_=xi_t[kt])
                TT(out=ta, in0=xr_t[kt], in1=xr_t[kt], op=ALU.mult)
                TT(out=sq32, in0=xi_t[kt], in1=xi_t[kt], op=ALU.mult)
                TT(out=sq32, in0=sq32, in1=ta, op=ALU.add)
                for q, src in enumerate([xr32, xi32, sq32]):
                    nc.tensor.matmul(pssum[q], ones32r, src,
                                     start=(kt == 0), stop=(kt == FT - 1))
            row = lnrow[0]
            lnrow[0] += 3
            for q in range(3):
                st_ = tpool.tile([1, NT], f32, tag="ln_st", bufs=2, name="lnst%d" % q)
                nc.scalar.copy(out=st_, in_=pssum[q])
                nc.sync.dma_start(out=io['scratch'][row + q, 0:NT][None, :], in_=st_)
            mr = p_bc.tile([P, NT], f32, tag="bc_mr")
            mi = p_bc.tile([P, NT], f32, tag="bc_mi")
            iv = p_bc.tile([P, NT], f32, tag="bc_iv")
            msq = p_bc.tile([P, NT], f32, tag="bc_msq")
            nc.sync.dma_start(out=mr, in_=io['scratch'][row + 0, 0:NT][None, :].to_broadcast([P, NT]))
            nc.sync.dma_start(out=mi, in_=io['scratch'][row + 1, 0:NT][None, :].to_broadcast([P, NT]))
            nc.sync.dma_start(out=iv, in_=io['scratch'][row + 2, 0:NT][None, :].to_broadcast([P, NT]))
            nc.scalar.mul(out=mr, in_=mr, mul=1.0 / D)
            nc.scalar.mul(out=mi, in_=mi, mul=1.0 / D)
            nc.scalar.mul(out=iv, in_=iv, mul=1.0 / D)
            TT(out=msq, in0=mr, in1=mr, op=ALU.mult)
            TT(out=iv, in0=iv, in1=msq, op=ALU.subtract)
            TT(out=msq, in0=mi, in1=mi, op=ALU.mult)
            TT(out=iv, in0=iv, in1=msq, op=ALU.subtract)
            nc.scalar.activation(out=iv, in_=iv, func=AF.Sqrt, bias=ceps)
            nc.vector.reciprocal(out=iv, in_=iv)
            gr_c, gi_c = lncols[ln + '_gr'], lncols[ln + '_gi']
            br_c, bi_c = lncols[ln + '_br'], lncols[ln + '_bi']
            for kt in range(FT):
                TT(out=xr_t[kt], in0=xr_t[kt], in1=mr, op=ALU.subtract)
                TT(out=xr_t[kt], in0=xr_t[kt], in1=iv, op=ALU.mult)   # nr
                TT(out=xi_t[kt], in0=xi_t[kt], in1=mi, op=ALU.subtract)
                TT(out=xi_t[kt], in0=xi_t[kt], in1=iv, op=ALU.mult)   # ni
                ta = tpool.tile([P, NT], f32, tag="ln_ta", bufs=2)
                tb = tpool.tile([P, NT], f32, tag="ln_tb", bufs=2)
                if hbufs == 0:
                    hr = hpool.tile([P, NT], f32r, name=htag + "r%d" % kt, uniquify=True)
                    hi = hpool.tile([P, NT], f32r, name=htag + "i%d" % kt, uniquify=True)
                else:
                    hr = hpool.tile([P, NT], f32r, tag=htag + "r", bufs=hbufs,
                                    name=htag + "hr", uniquify=True)
                    hi = hpool.tile([P, NT], f32r, tag=htag + "i", bufs=hbufs,
                                    name=htag + "hi", uniquify=True)
                TS(out=ta, in0=xr_t[kt], scalar1=gr_c[:, kt:kt + 1], scalar2=None, op0=ALU.mult)
                TS(out=tb, in0=xi_t[kt], scalar1=gi_c[:, kt:kt + 1], scalar2=None, op0=ALU.mult)
                TT(out=ta, in0=ta, in1=tb, op=ALU.subtract)
                TS(out=hr, in0=ta, scalar1=br_c[:, kt:kt + 1], scalar2=None, op0=ALU.add)
                TS(out=ta, in0=xr_t[kt], scalar1=gi_c[:, kt:kt + 1], scalar2=None, op0=ALU.mult)
                TS(out=tb, in0=xi_t[kt], scalar1=gr_c[:, kt:kt + 1], scalar2=None, op0=ALU.mult)
                TT(out=ta, in0=ta, in1=tb, op=ALU.add)
                TS(out=hi, in0=ta, scalar1=bi_c[:, kt:kt + 1], scalar2=None, op0=ALU.add)
                dst_r.append(hr)
                dst_i.append(hi)

        def load_x_chunk(name_r, name_i, csl, tpool):
            xr_t, xi_t = [], []
            for kt in range(FT):
                a = tpool.tile([P, 512], f32, tag="x_ldr", bufs=4)
                b_ = tpool.tile([P, 512], f32, tag="x_ldi", bufs=4)
                nc.sync.dma_start(out=a, in_=io[name_r][kt * P:(kt + 1) * P, csl])
                nc.sync.dma_start(out=b_, in_=io[name_i][kt * P:(kt + 1) * P, csl])
                xr_t.append(a)
                xi_t.append(b_)
            return xr_t, xi_t

        def cplx_mm(ps_r, ps_i, w3, kt, nkt, rhs_r, rhs_i, msl):
            wr, wi, wineg = w3
            first, last = kt == 0, kt == nkt - 1
            nc.tensor.matmul(ps_r, wr[:, msl], rhs_r, start=first, stop=False)
            nc.tensor.matmul(ps_r, wineg[:, msl], rhs_i, start=False, stop=last)
            nc.tensor.matmul(ps_i, wi[:, msl], rhs_r, start=first, stop=False)
            nc.tensor.matmul(ps_i, wr[:, msl], rhs_i, start=False, stop=last)

        def rope_apply(dst, src_ps, costab, sintab, bias_col, tpool):
            NT = 512
            pre = tpool.tile([P, NT], f32, tag="rope_pre", bufs=2)
            tmp = tpool.tile([P, NT], f32, tag="rope_tmp", bufs=2)
            TS(out=pre, in0=src_ps, scalar1=bias_col, scalar2=None, op0=ALU.add)
            TT(out=dst, in0=pre, in1=costab, op=ALU.mult)
            for g in range(2):
                # sin/cos tables repeat with period 32 partitions, so the
                # rotate-half pair (p <-> p+32) can read the table at the
                # SOURCE partition (walrus: SB inputs must share base part).
                b0 = g * 64
                TT(out=tmp[b0:b0 + 32, :], in0=pre[b0 + 32:b0 + 64, :],
                   in1=sintab[b0 + 32:b0 + 64, :], op=ALU.mult)
                TT(out=dst[b0:b0 + 32, :], in0=dst[b0:b0 + 32, :],
                   in1=tmp[b0:b0 + 32, :], op=ALU.subtract)
                TT(out=tmp[b0 + 32:b0 + 64, :], in0=pre[b0:b0 + 32, :],
                   in1=sintab[b0:b0 + 32, :], op=ALU.mult)
                TT(out=dst[b0 + 32:b0 + 64, :], in0=dst[b0 + 32:b0 + 64, :],
                   in1=tmp[b0 + 32:b0 + 64, :], op=ALU.add)

        # ================= attention-lifetime tensors =================
        es_qkv = ExitStack()
        p_qk = es_qkv.enter_context(tc.tile_pool(name="p_qk", bufs=1))
        q_rot_r = [p_qk.tile([P, TQ], f32r, name='qrr%d' % ot) for ot in range(FT)]
        q_rot_i = [p_qk.tile([P, TQ], f32r, name='qri%d' % ot) for ot in range(FT)]
        k_rot_r = [p_qk.tile([P, S], f32r, name='krr%d' % ot) for ot in range(FT)]
        k_rot_i = [p_qk.tile([P, S], f32r, name='kri%d' % ot) for ot in range(FT)]
        vaug = [p_qk.tile([P, H, 129], bf16, name='vaug%d' % t) for t in range(8)]

        # ================= Phase Q =================
        with tc.tile_pool(name="p_q", bufs=1) as p_q, \
             tc.tile_pool(name="p_qc", bufs=1) as p_qc:
            cq = p_qc.tile([P, TQ], f32, name='cq')
            sq_t = p_qc.tile([P, TQ], f32, name='sq_t')
            nc.sync.dma_start(out=cq, in_=io['cosq'][:])
            nc.sync.dma_start(out=sq_t, in_=io['sinq'][:])
            xr_t, xi_t = load_x_chunk('xq_r', 'xq_i', slice(0, TQ), p_q)
            hq_r, hq_i = [], []
            ln_chunk(xr_t, xi_t, 'ln1', hq_r, hq_i, p_q, "hq", p_q)
            wq = proj_weights('q')
            bq_r, bq_i = bias_cols['q']
            for ot in range(FT):
                msl = slice(ot * P, (ot + 1) * P)
                pr = ps.tile([P, TQ], f32, tag="ps_rot")
                pi = ps.tile([P, TQ], f32, tag="ps_rot")
                for kt in range(FT):
                    cplx_mm(pr, pi, wq[kt], kt, FT, hq_r[kt], hq_i[kt], msl)
                rope_apply(q_rot_r[ot], pr, cq, sq_t, bq_r[:, ot:ot + 1], p_q)
                rope_apply(q_rot_i[ot], pi, cq, sq_t, bq_i[:, ot:ot + 1], p_q)

        # ================= Phase K/V =================
        with tc.tile_pool(name="p_kv", bufs=1) as p_kv, \
             tc.tile_pool(name="p_kc", bufs=1) as p_kc:
            ck = p_kc.tile([P, S], f32, name='ck')
            sk_t = p_kc.tile([P, S], f32, name='sk_t')
            nc.sync.dma_start(out=ck, in_=io['cosk'][:])
            nc.sync.dma_start(out=sk_t, in_=io['sink'][:])
            for va in vaug:
                nc.vector.memset(va[:, :, 64:65], 1.0)

            hf_r, hf_i = [], []
            for ch in range(2):
                csl = slice(ch * 512, (ch + 1) * 512)
                xr_t, xi_t = load_x_chunk('xf_r', 'xf_i', csl, p_kv)
                ln_chunk(xr_t, xi_t, 'ln1', hf_r, hf_i, p_kv, "hf", p_kv, hbufs=8)
            wk = proj_weights('k')
            bk_r, bk_i = bias_cols['k']
            for ch in range(2):
                csl = slice(ch * 512, (ch + 1) * 512)
                for ot in range(FT):
                    msl = slice(ot * P, (ot + 1) * P)
                    pr = ps.tile([P, 512], f32, tag="ps_rot")
                    pi = ps.tile([P, 512], f32, tag="ps_rot")
                    for kt in range(FT):
                        cplx_mm(pr, pi, wk[kt], kt, FT, hf_r[ch * 4 + kt], hf_i[ch * 4 + kt], msl)
                    rope_apply(k_rot_r[ot][:, csl], pr, ck[:, csl], sk_t[:, csl],
                               bk_r[:, ot:ot + 1], p_kv)
                    rope_apply(k_rot_i[ot][:, csl], pi, ck[:, csl], sk_t[:, csl],
                               bk_i[:, ot:ot + 1], p_kv)
            wv = proj_weights('v')
            for tkt in range(8):
                ch, tk4 = tkt // 4, tkt % 4
                tsl = slice(tk4 * P, (tk4 + 1) * P)
                pr = ps.tile([P, D], f32, tag="ps_rot")
                pi = ps.tile([P, D], f32, tag="ps_rot")
                for kt in range(FT):
                    first, last = kt == 0, kt == FT - 1
                    hfr_t, hfi_t = hf_r[ch * 4 + kt], hf_i[ch * 4 + kt]
                    nc.tensor.matmul(pr, hfr_t[:, tsl], wv[kt][0], start=first, stop=False)
                    nc.tensor.matmul(pr, hfi_t[:, tsl], wv[kt][2], start=False, stop=last)
                    nc.tensor.matmul(pi, hfr_t[:, tsl], wv[kt][1], start=first, stop=False)
                    nc.tensor.matmul(pi, hfi_t[:, tsl], wv[kt][0], start=False, stop=last)
                nc.vector.tensor_copy(out=vaug[tkt][:, :, 0:64],
                                      in_=pr.rearrange("p (h d) -> p h d", h=H))
                nc.vector.tensor_copy(out=vaug[tkt][:, :, 65:129],
                                      in_=pi.rearrange("p (h d) -> p h d", h=H))

        # ================= Phase attention =================
        es_attn = ExitStack()
        p_am = es_attn.enter_context(tc.tile_pool(name="p_am", bufs=1, side='right'))
        attn_r = [p_am.tile([P, TQ], f32r, name='attnr%d' % ot) for ot in range(FT)]
        attn_i = [p_am.tile([P, TQ], f32r, name='attni%d' % ot) for ot in range(FT)]
        with tc.tile_pool(name="p_at", bufs=1) as p_at, \
             tc.tile_pool(name="p_pt", bufs=4) as p_pt:
            mask_t = []
            for tkt in range(8):
                m = p_at.tile([P, TQ], f32, name='mask%d' % tkt)
                nc.sync.dma_start(out=m, in_=io['maskT'][tkt * P:(tkt + 1) * P, :])
                mask_t.append(m)
            for h in range(H):
                ot, prow = h // 2, 64 * (h % 2)
                qr_h = q_rot_r[ot][prow:prow + 64, :]
                qi_h = q_rot_i[ot][prow:prow + 64, :]
                po_r = ps_at.tile([65, TQ], f32, tag="at_r")
                po_i = ps_at.tile([64, TQ], f32, tag="at_i")
                for tkt in range(8):
                    tsl = slice(tkt * P, (tkt + 1) * P)
                    pst = ps.tile([P, TQ], f32, tag="ps_rot")
                    nc.tensor.matmul(pst, k_rot_r[ot][prow:prow + 64, tsl], qr_h,
                                     start=True, stop=False)
                    nc.tensor.matmul(pst, k_rot_i[ot][prow:prow + 64, tsl], qi_h,
                                     start=False, stop=True)
                    pe_ = p_pt.tile([P, TQ], f32, tag="p_exp")
                    p32 = p_pt.tile([P, TQ], bf16, tag="p_32r")
                    nc.scalar.activation(out=pe_, in_=pst, func=AF.Exp, scale=SCALE)
                    TT(out=p32, in0=pe_, in1=mask_t[tkt], op=ALU.mult)
                    nc.tensor.matmul(po_r, vaug[tkt][:, h, 0:65], p32,
                                     start=(tkt == 0), stop=(tkt == 7))
                    nc.tensor.matmul(po_i, vaug[tkt][:, h, 65:129], p32,
                                     start=(tkt == 0), stop=(tkt == 7))
                dnst = p_pt.tile([1, TQ], f32, tag="dn_st")
                nc.scalar.copy(out=dnst, in_=po_r[64:65, :])
                nc.sync.dma_start(out=io['scratch'][30, 0:TQ][None, :], in_=dnst)
                rec = p_bc.tile([64, TQ], f32, tag="bc_rec")
                nc.sync.dma_start(out=rec,
                                  in_=io['scratch'][30, 0:TQ][None, :].to_broadcast([64, TQ]))
                nc.vector.reciprocal(out=rec, in_=rec)
                TT(out=attn_r[ot][prow:prow + 64, :], in0=po_r[0:64, :], in1=rec, op=ALU.mult)
                TT(out=attn_i[ot][prow:prow + 64, :], in0=po_i[0:64, :], in1=rec, op=ALU.mult)
                bvr_c, bvi_c = bias_cols['v']
                TS(out=attn_r[ot][prow:prow + 64, :], in0=attn_r[ot][prow:prow + 64, :],
                   scalar1=bvr_c[prow:prow + 64, ot:ot + 1], scalar2=None, op0=ALU.add)
                TS(out=attn_i[ot][prow:prow + 64, :], in0=attn_i[ot][prow:prow + 64, :],
                   scalar1=bvi_c[prow:prow + 64, ot:ot + 1], scalar2=None, op0=ALU.add)
        es_qkv.close()   # free q/k/vaug

        # ================= Phase O-proj + residual + LN2 =================
        es_keep = ExitStack()
        p_keep = es_keep.enter_context(tc.tile_pool(name="p_keep", bufs=1))
        res_r = [p_keep.tile([P, TQ], f32, name='resr%d' % ot) for ot in range(FT)]
        res_i = [p_keep.tile([P, TQ], f32, name='resi%d' % ot) for ot in range(FT)]
        h2_r, h2_i = [], []
        with tc.tile_pool(name="p_o", bufs=1) as p_o:
            wo = proj_weights('o')
            bo_r, bo_i = bias_cols['o']
            xr_t, xi_t = load_x_chunk('xq_r', 'xq_i', slice(0, TQ), p_o)
            for ot in range(FT):
                msl = slice(ot * P, (ot + 1) * P)
                pr = ps.tile([P, TQ], f32, tag="ps_rot")
                pi = ps.tile([P, TQ], f32, tag="ps_rot")
                for kt in range(FT):
                    cplx_mm(pr, pi, wo[kt], kt, FT, attn_r[kt], attn_i[kt], msl)
                TS(out=res_r[ot], in0=pr, scalar1=bo_r[:, ot:ot + 1], scalar2=None, op0=ALU.add)
                TT(out=res_r[ot], in0=res_r[ot], in1=xr_t[ot], op=ALU.add)
                TS(out=res_i[ot], in0=pi, scalar1=bo_i[:, ot:ot + 1], scalar2=None, op0=ALU.add)
                TT(out=res_i[ot], in0=res_i[ot], in1=xi_t[ot], op=ALU.add)
            es_attn.close()   # free attn tiles
            cr_t, ci_t = [], []
            for ot in range(FT):
                cr = p_o.tile([P, TQ], f32, tag="ln_cr", bufs=5)
                ci = p_o.tile([P, TQ], f32, tag="ln_ci", bufs=5)
                nc.vector.tensor_copy(out=cr, in_=res_r[ot])
                nc.vector.tensor_copy(out=ci, in_=res_i[ot])
                cr_t.append(cr)
                ci_t.append(ci)
            ln_chunk(cr_t, ci_t, 'ln2', h2_r, h2_i, p_keep, "h2", p_o, hbufs=0)

        # ================= Phase FFN (gate/up/gating + interleaved down) =================
        acc_r = [p_keep.tile([P, TQ], f32, name='accr%d' % ot) for ot in range(FT)]
        acc_i = [p_keep.tile([P, TQ], f32, name='acci%d' % ot) for ot in range(FT)]
        with tc.tile_pool(name="p_f", bufs=1) as p_f, \
             tc.tile_pool(name="p_fh", bufs=1) as p_fh:
            for grp in range(4):
                hids = []
                for j in range(4):
                    ot = grp * 4 + j
                    osl = slice(ot * P, (ot + 1) * P)

                    def ffn_w(lmT, phT, tag):
                        lm = p_f.tile([P, FT, P], f32, tag=tag + "_lm")
                        ph = p_f.tile([P, FT, P], f32, tag=tag + "_ph")
                        for kt in range(FT):
                            nc.sync.dma_start(out=lm[:, kt, :], in_=lmT[kt * P:(kt + 1) * P, osl])
                            nc.sync.dma_start(out=ph[:, kt, :], in_=phT[kt * P:(kt + 1) * P, osl])
                        ssq = p_f.tile([P, FT, P], f32, tag=tag + "_ssq")
                        cosv = p_f.tile([P, FT, P], f32, tag=tag + "_cosv")
                        wr = p_f.tile([P, FT, P], f32r, tag=tag + "_wr", bufs=2)
                        wi = p_f.tile([P, FT, P], f32r, tag=tag + "_wi", bufs=2)
                        wineg = p_f.tile([P, FT, P], f32r, tag=tag + "_wineg", bufs=2)
                        nc.scalar.activation(out=lm, in_=lm, func=AF.Exp)
                        nc.scalar.activation(out=ph, in_=ph, func=AF.Sin, scale=0.5)
                        TT(out=ssq, in0=ph, in1=ph, op=ALU.mult)
                        TS(out=cosv, in0=ssq, scalar1=-2.0, scalar2=1.0, op0=ALU.mult, op1=ALU.add)
                        nc.scalar.activation(out=ssq, in_=ssq, func=AF.Sqrt, scale=-4.0, bias=c4)
                        TT(out=ph, in0=ph, in1=ssq, op=ALU.mult)
                        TT(out=wr, in0=lm, in1=cosv, op=ALU.mult)
                        TT(out=wi, in0=lm, in1=ph, op=ALU.mult)
                        TS(out=wineg, in0=wi, scalar1=-1.0, scalar2=None, op0=ALU.mult)
                        return wr, wi, wineg

                    gw = ffn_w(io['g_lmT'], io['g_phT'], "gw")
                    uw = ffn_w(io['u_lmT'], io['u_phT'], "uw")
                    pgr = ps.tile([P, TQ], f32, tag="ps_rot")
                    pgi = ps.tile([P, TQ], f32, tag="ps_rot")
                    for kt in range(FT):
                        cplx_mm(pgr, pgi, (gw[0][:, kt, :], gw[1][:, kt, :], gw[2][:, kt, :]),
                                kt, FT, h2_r[kt], h2_i[kt], slice(0, P))
                    pur = ps.tile([P, TQ], f32, tag="ps_rot")
                    pui = ps.tile([P, TQ], f32, tag="ps_rot")
                    for kt in range(FT):
                        cplx_mm(pur, pui, (uw[0][:, kt, :], uw[1][:, kt, :], uw[2][:, kt, :]),
                                kt, FT, h2_r[kt], h2_i[kt], slice(0, P))
                    sq1 = p_f.tile([P, TQ], f32, tag="f_sq1", bufs=2)
                    sq2 = p_f.tile([P, TQ], f32, tag="f_sq2")
                    gar = p_f.tile([P, TQ], f32, tag="f_gar", bufs=2)
                    gai = p_f.tile([P, TQ], f32, tag="f_gai", bufs=2)
                    nc.scalar.activation(out=sq1, in_=pgr, func=AF.Square)
                    nc.scalar.activation(out=sq2, in_=pgi, func=AF.Square)
                    TT(out=sq1, in0=sq1, in1=sq2, op=ALU.add)
                    nc.scalar.activation(out=sq1, in_=sq1, func=AF.Sqrt)
                    nc.scalar.activation(out=sq1, in_=sq1, func=AF.Sigmoid)
                    TT(out=gar, in0=pgr, in1=sq1, op=ALU.mult)
                    TT(out=gai, in0=pgi, in1=sq1, op=ALU.mult)
                    t1 = p_f.tile([P, TQ], f32, tag="f_t1")
                    t2 = p_f.tile([P, TQ], f32, tag="f_t2")
                    hr = p_fh.tile([P, TQ], f32r, tag="hidr", bufs=5)
                    hi = p_fh.tile([P, TQ], f32r, tag="hidi", bufs=5)
                    TT(out=t1, in0=gar, in1=pur, op=ALU.mult)
                    TT(out=t2, in0=gai, in1=pui, op=ALU.mult)
                    TT(out=hr, in0=t1, in1=t2, op=ALU.subtract)
                    TT(out=t1, in0=gar, in1=pui, op=ALU.mult)
                    TT(out=t2, in0=gai, in1=pur, op=ALU.mult)
                    TT(out=hi, in0=t1, in1=t2, op=ALU.add)
                    hids.append((hr, hi))
                # ---- down-projection for this group of 4 hid k-tiles ----
                dws = [prep_w(io['d_lmT'][kt * P:(kt + 1) * P, :],
                              io['d_phT'][kt * P:(kt + 1) * P, :], D)
                       for kt in range(grp * 4, grp * 4 + 4)]
                for ot in range(FT):
                    msl = slice(ot * P, (ot + 1) * P)
                    pr = ps.tile([P, TQ], f32, tag="ps_rot")
                    pi = ps.tile([P, TQ], f32, tag="ps_rot")
                    for j in range(4):
                        cplx_mm(pr, pi, dws[j], j, 4, hids[j][0], hids[j][1], msl)
                    if grp == 0:
                        nc.vector.tensor_copy(out=acc_r[ot], in_=pr)
                        nc.vector.tensor_copy(out=acc_i[ot], in_=pi)
                    else:
                        TT(out=acc_r[ot], in0=acc_r[ot], in1=pr, op=ALU.add)
                        TT(out=acc_i[ot], in0=acc_i[ot], in1=pi, op=ALU.add)
        for ot in range(FT):
            TT(out=acc_r[ot], in0=acc_r[ot], in1=res_r[ot], op=ALU.add)
            TT(out=acc_i[ot], in0=acc_i[ot], in1=res_i[ot], op=ALU.add)
            nc.sync.dma_start(out=io['out_r'][ot * P:(ot + 1) * P, :], in_=acc_r[ot])
            nc.sync.dma_start(out=io['out_i'][ot * P:(ot + 1) * P, :], in_=acc_i[ot])
        es_keep.close()


def _build_module(n_iters=1):
    import concourse.tile as tile
    from concourse import bacc, mybir

    f32 = mybir.dt.float32
    nc = bacc.Bacc(None, target_bir_lowering=False, debug=False)
    with tile.TileContext(nc) as tc:
        with tc.tile_pool(name="dram", bufs=1, space="DRAM") as dram:
            io = {}

            def din(name, shape):
                io[name] = dram.tile(shape, f32, kind='ExternalInput', name=name, uniquify=False)

            din('xq_r', [D, TQ]); din('xq_i', [D, TQ])
            din('xf_r', [D, S]); din('xf_i', [D, S])
            for nm in ['q', 'k', 'v', 'o']:
                din(nm + '_lmT', [D, D]); din(nm + '_phT', [D, D])
                din(nm + '_bm', [D]); din(nm + '_bp', [D])
            din('g_lmT', [D, HID]); din('g_phT', [D, HID])
            din('u_lmT', [D, HID]); din('u_phT', [D, HID])
            din('d_lmT', [HID, D]); din('d_phT', [HID, D])
            for ln in ['ln1', 'ln2']:
                for q in ['gr', 'gi', 'br', 'bi']:
                    din(ln + '_' + q, [D])
            din('cosq', [P, TQ]); din('sinq', [P, TQ])
            din('cosk', [P, S]); din('sink', [P, S])
            din('maskT', [S, TQ])
            io['out_r'] = dram.tile([D, TQ], f32, kind='ExternalOutput', name='out_r', uniquify=False)
            io['out_i'] = dram.tile([D, TQ], f32, kind='ExternalOutput', name='out_i', uniquify=False)
            io['scratch'] = dram.tile([32, S], f32, name='scratch', uniquify=False)

            if n_iters == 1:
                _emit_body(nc, tc, io)
            else:
                with tc.For_i(0, n_iters, 1):
                    _emit_body(nc, tc, io)
    nc.compile()
    return nc


def _host_inputs(x_real, x_imag, full, core):
    b, hh = core // 2, core % 2
    qs = hh * TQ
    m = {}
    m['xq_r'] = np.ascontiguousarray(x_real[b, qs:qs + TQ, :].T)
    m['xq_i'] = np.ascontiguousarray(x_imag[b, qs:qs + TQ, :].T)
    m['xf_r'] = np.ascontiguousarray(x_real[b].T)
    m['xf_i'] = np.ascontiguousarray(x_imag[b].T)
    for nm in ['q', 'k', 'v', 'o']:
        m[nm + '_lmT'] = np.ascontiguousarray(full[nm + '_lm'].T)
        m[nm + '_phT'] = np.ascontiguousarray(full[nm + '_ph'].T)
        m[nm + '_bm'] = full[nm + '_bm']
        m[nm + '_bp'] = full[nm + '_bp']
    m['g_lmT'] = np.ascontiguousarray(full['gate_lm'].T)
    m['g_phT'] = np.ascontiguousarray(full['gate_ph'].T)
    m['u_lmT'] = np.ascontiguousarray(full['up_lm'].T)
    m['u_phT'] = np.ascontiguousarray(full['up_ph'].T)
    m['d_lmT'] = np.ascontiguousarray(full['down_lm'].T)
    m['d_phT'] = np.ascontiguousarray(full['down_ph'].T)
    for ln in ['ln1', 'ln2']:
        for q in ['gr', 'gi', 'br', 'bi']:
            m[ln + '_' + q] = full[ln + '_' + q]
    invf = 1.0 / (10000.0 ** (np.arange(0, HD, 2, dtype=np.float64) / HD))
    pidx = (np.arange(P) % HD) % 32
    angq = np.arange(qs, qs + TQ, dtype=np.float64)[None, :] * invf[pidx][:, None]
    angk = np.arange(S, dtype=np.float64)[None, :] * invf[pidx][:, None]
    m['cosq'] = np.cos(angq).astype(np.float32)
    m['sinq'] = np.sin(angq).astype(np.float32)
    m['cosk'] = np.cos(angk).astype(np.float32)
    m['sink'] = np.sin(angk).astype(np.float32)
    tk = np.arange(S)[:, None]
    tq = (qs + np.arange(TQ))[None, :]
    m['maskT'] = (tq >= tk).astype(np.float32)
    return m


def kernel(**inputs):
    from concourse.bass_utils import run_bass_kernel_spmd

    full = {k: np.asarray(v, dtype=np.float32) for k, v in inputs.items()}
    x_real, x_imag = full['x_real'], full['x_imag']

    if 'mod' not in _CACHE:
        _CACHE['mod'] = _build_module(1)
    nc = _CACHE['mod']

    in_maps = [_host_inputs(x_real, x_imag, full, c) for c in range(NCORES)]
    res = run_bass_kernel_spmd(nc, in_maps, core_ids=list(range(NCORES)), trace=False)

    out = np.empty((2, B, S, D), dtype=np.float32)
    for c in range(NCORES):
        b, hh = c // 2, c % 2
        qs = hh * TQ
        out[0, b, qs:qs + TQ, :] = res.results[c]['out_r'].T
        out[1, b, qs:qs + TQ, :] = res.results[c]['out_i'].T
    return out
